# revision 12
# baseline (speedup 1.0000x reference)
"""RNN-T joint network kernel for 8 Trainium2 NeuronCores.

out[b,t,u,:] = W_out @ tanh(W_enc @ enc[b,t] + b_enc + W_dec @ dec[b,u]) + b_out

Sharding: data-parallel over B (8 batches -> 8 cores), weights replicated.

Per-core device pipeline (b fixed, TU = T*U = 20000 joint positions):
  1. fp32 matmuls:  enc_projT[j,t] (J=512 on partitions, 4 j-tiles),
                    dec_projT[j,u]
  2. DVE broadcast-add (stride-0 APs) + ACT tanh -> bf16 jointT[j, t*U+u],
     produced in t-chunks of 32 (3200 columns)
  3. big matmul per 128-wide tu-tile: stationary = jointT slice [128j,128tu],
     moving = W_outT [128j, 512v] bf16, accumulate 4 j-tiles into fp32 PSUM
  4. PSUM->SBUF copies (alternating DVE/ACT), staged 5-tile (2.5MB) DMA stores

b_out is added on the host (per-column bias on device would double DVE cost);
weight transposes are done on the host (numpy) - they are layout prep, not
FLOPs.
"""

import numpy as np

B, T, U = 8, 200, 100
D = 512      # d_enc == d_dec
J = 512      # joint dim
V = 1024     # vocab
TU = T * U   # 20000 joint positions per core
TCH = 32     # t values per chunk (3200 columns; 25 full 128-wide tu tiles)
NJ = J // 128   # 4 j partition-tiles
ND = D // 128   # 4 d partition-tiles
GRP = 5      # tu-tiles per staged output DMA (5*128*1024*4B = 2.5MB)

_CACHE = {}


def _build():
    import concourse.bass as bass
    import concourse.mybir as mybir
    from concourse import tile

    f32 = mybir.dt.float32
    bf16 = mybir.dt.bfloat16
    AF = mybir.ActivationFunctionType
    ALU = mybir.AluOpType

    nc = bass.Bass()

    encT_d = nc.dram_tensor("encT", [D, T], f32, kind="ExternalInput")
    decT_d = nc.dram_tensor("decT", [D, U], f32, kind="ExternalInput")
    wencT_d = nc.dram_tensor("wencT", [D, J], f32, kind="ExternalInput")
    wdecT_d = nc.dram_tensor("wdecT", [D, J], f32, kind="ExternalInput")
    woutT_d = nc.dram_tensor("woutT", [J, V], bf16, kind="ExternalInput")
    benc_d = nc.dram_tensor("benc", [J, 1], f32, kind="ExternalInput")
    out_d = nc.dram_tensor("out", [TU, V], f32, kind="ExternalOutput")

    with tile.TileContext(nc) as tc:
        with (
            tc.tile_pool(name="const", bufs=1) as cpool,
            tc.tile_pool(name="proj", bufs=1) as ppool,
            tc.tile_pool(name="pre", bufs=3) as prepool,
            tc.tile_pool(name="joint", bufs=8) as jpool,
            tc.tile_pool(name="stage", bufs=2) as stpool,
            tc.tile_pool(name="pspro", bufs=1, space="PSUM") as pspro,
            tc.tile_pool(name="psmain", bufs=3, space="PSUM") as psmain,
        ):
            # ---- constant loads -------------------------------------------
            wenc_sb = [cpool.tile([128, J], f32, tag=f"wenc{d}", name=f"wenc{d}") for d in range(ND)]
            wdec_sb = [cpool.tile([128, J], f32, tag=f"wdec{d}", name=f"wdec{d}") for d in range(ND)]
            wout_sb = [cpool.tile([128, V], bf16, tag=f"wout{j}", name=f"wout{j}") for j in range(NJ)]
            enc_sb = [cpool.tile([128, T], f32, tag=f"enc{d}", name=f"enc{d}") for d in range(ND)]
            dec_sb = [cpool.tile([128, U], f32, tag=f"dec{d}", name=f"dec{d}") for d in range(ND)]
            benc_sb = [cpool.tile([128, 1], f32, tag=f"benc{j}", name=f"benc{j}") for j in range(NJ)]
            for d in range(ND):
                sl = slice(d * 128, (d + 1) * 128)
                nc.sync.dma_start(wenc_sb[d][:], wencT_d[sl, :])
                nc.sync.dma_start(wdec_sb[d][:], wdecT_d[sl, :])
                nc.sync.dma_start(enc_sb[d][:], encT_d[sl, :])
                nc.sync.dma_start(dec_sb[d][:], decT_d[sl, :])
            for j in range(NJ):
                sl = slice(j * 128, (j + 1) * 128)
                nc.sync.dma_start(wout_sb[j][:], woutT_d[sl, :])
                nc.sync.dma_start(benc_sb[j][:], benc_d[sl, :])

            # ---- small projections (fp32) ---------------------------------
            # enc_projT[j,t] = sum_d W_enc[j,d] * enc[t,d] + b_enc[j]
            enc_proj = [ppool.tile([128, T], f32, tag=f"ep{j}", name=f"ep{j}") for j in range(NJ)]
            dec_proj = [ppool.tile([128, U], f32, tag=f"dp{j}", name=f"dp{j}") for j in range(NJ)]
            for j in range(NJ):
                ps = pspro.tile([128, T], f32, tag="pse")
                for d in range(ND):
                    nc.tensor.matmul(
                        ps[:],
                        wenc_sb[d][:, j * 128:(j + 1) * 128],
                        enc_sb[d][:],
                        start=(d == 0),
                        stop=(d == ND - 1),
                    )
                nc.scalar.activation(enc_proj[j][:], ps[:], AF.Identity, bias=benc_sb[j][:])
            for j in range(NJ):
                ps = pspro.tile([128, U], f32, tag="psd")
                for d in range(ND):
                    nc.tensor.matmul(
                        ps[:],
                        wdec_sb[d][:, j * 128:(j + 1) * 128],
                        dec_sb[d][:],
                        start=(d == 0),
                        stop=(d == ND - 1),
                    )
                nc.vector.tensor_copy(dec_proj[j][:], ps[:])

            # ---- main loop over t-chunks ----------------------------------
            tile_ctr = 0
            for t0 in range(0, T, TCH):
                nt = min(TCH, T - t0)
                cols = nt * U
                tu0 = t0 * U

                # jointT[j, t*U+u] = tanh(enc_projT[j,t] + dec_projT[j,u])
                joints = []
                for j in range(NJ):
                    pre = prepool.tile([128, cols], f32, tag="pre")
                    nc.vector.tensor_tensor(
                        pre.rearrange("p (t u) -> p t u", u=U),
                        enc_proj[j][:, t0:t0 + nt].unsqueeze(2).broadcast_to([128, nt, U]),
                        dec_proj[j][:, :].unsqueeze(1).broadcast_to([128, nt, U]),
                        ALU.add,
                    )
                    jt = jpool.tile([128, cols], bf16, tag="joint")
                    nc.scalar.activation(jt[:], pre[:], AF.Tanh)
                    joints.append(jt)

                # out[tu, v] = sum_j jointT[j, tu] * W_outT[j, v]
                offs = [(c, min(128, cols - c)) for c in range(0, cols, 128)]
                idx = 0
                while idx < len(offs):
                    grp = []
                    while (idx < len(offs) and len(grp) < GRP
                           and offs[idx][1] == 128):
                        grp.append(offs[idx])
                        idx += 1
                    if not grp:        # single partial-width tail tile
                        grp = [offs[idx]]
                        idx += 1
                    st = stpool.tile([128, GRP, V], f32, tag="stage")
                    for g, (c, w) in enumerate(grp):
                        psA = psmain.tile([128, 512], f32, tag="psA")
                        psB = psmain.tile([128, 512], f32, tag="psB")
                        for j in range(NJ):
                            nc.tensor.matmul(
                                psA[0:w, :], joints[j][:, c:c + w],
                                wout_sb[j][:, 0:512],
                                start=(j == 0), stop=(j == NJ - 1),
                            )
                        for j in range(NJ):
                            nc.tensor.matmul(
                                psB[0:w, :], joints[j][:, c:c + w],
                                wout_sb[j][:, 512:V],
                                start=(j == 0), stop=(j == NJ - 1),
                            )
                        # PSUM -> SBUF stage; alternate engines to balance load
                        nc.vector.tensor_copy(st[0:w, g, 0:512], psA[0:w, :])
                        if tile_ctr % 2 == 0:
                            nc.scalar.activation(st[0:w, g, 512:V], psB[0:w, :], AF.Copy)
                        else:
                            nc.vector.tensor_copy(st[0:w, g, 512:V], psB[0:w, :])
                        tile_ctr += 1
                    w0 = grp[0][1]
                    if w0 == 128:
                        G = len(grp)
                        r0 = tu0 + grp[0][0]
                        dst = out_d[r0:r0 + G * 128, :].rearrange(
                            "(g p) v -> p g v", p=128)
                        nc.sync.dma_start(dst, st[:, 0:G, :])
                    else:
                        c, w = grp[0]
                        r0 = tu0 + c
                        nc.sync.dma_start(out_d[r0:r0 + w, :], st[0:w, 0, :])

    _fix_matmul_waits(nc)
    return nc


def _fix_matmul_waits(nc):
    """TRN2 TPB instructions take at most 1 semaphore wait (EventSemaphore: 2),
    but Tile emits up to 4 on one instruction. For each saturated compute
    instruction, park the excess waits on EventSemaphore instructions inserted
    immediately before it on the same engine (no reordering, so the schedule's
    correctness argument is untouched)."""
    import concourse.mybir as mybir

    capped = (
        mybir.InstMatmult, mybir.InstLdweights, mybir.InstActivation,
        mybir.InstTensorTensor, mybir.InstTensorCopy, mybir.InstMemset,
        mybir.InstTensorReduce, mybir.InstDMACopy, mybir.InstDrain,
    )
    _n = [0]
    for f in nc.m.functions:
        for blk in f.blocks:
            fixups = []
            for inst in blk.instructions:
                if not isinstance(inst, capped):
                    continue
                si = inst.sync_info
                if si is None or len(si.on_wait) <= 1:
                    continue
                waits = list(si.on_wait)
                fixups.append((inst, waits[:-1]))
                si.on_wait = waits[-1:]
            for inst, excess in fixups:
                idx = blk.instructions.index(inst)
                for i in range(0, len(excess), 2):
                    ev = mybir.InstEventSemaphore(
                        name=f"waitfix-{_n[0]}",
                        engine=inst.engine,
                        sync_info=mybir.SyncInfo(
                            on_wait=excess[i:i + 2], on_update=[]),
                    )
                    _n[0] += 1
                    blk.instructions.insert(idx, ev)
                    idx += 1


def _get_nc():
    if "nc" not in _CACHE:
        _CACHE["nc"] = _build()
    return _CACHE["nc"]


def _prep_in_maps(inputs):
    import ml_dtypes

    enc_out = np.asarray(inputs["enc_out"], np.float32)   # (B,T,1,D)
    dec_out = np.asarray(inputs["dec_out"], np.float32)   # (B,1,U,D)
    W_enc = np.asarray(inputs["W_enc"], np.float32)       # (J,D)
    W_dec = np.asarray(inputs["W_dec"], np.float32)       # (J,D)
    W_out = np.asarray(inputs["W_out"], np.float32)       # (V,J)
    b_enc = np.asarray(inputs["b_enc"], np.float32)       # (J,)

    encT = np.ascontiguousarray(enc_out[:, :, 0, :].transpose(0, 2, 1))  # (B,D,T)
    decT = np.ascontiguousarray(dec_out[:, 0, :, :].transpose(0, 2, 1))  # (B,D,U)
    wencT = np.ascontiguousarray(W_enc.T)                                # (D,J)
    wdecT = np.ascontiguousarray(W_dec.T)                                # (D,J)
    woutT = np.ascontiguousarray(W_out.T).astype(ml_dtypes.bfloat16)     # (J,V)
    benc = np.ascontiguousarray(b_enc.reshape(J, 1))

    return [
        dict(encT=encT[b], decT=decT[b], wencT=wencT, wdecT=wdecT,
             woutT=woutT, benc=benc)
        for b in range(B)
    ]


def _run(inputs, trace=False):
    from concourse.bass_utils import run_bass_kernel_spmd

    in_maps = _prep_in_maps(inputs)
    nc = _get_nc()
    res = run_bass_kernel_spmd(nc, in_maps, list(range(B)), trace=trace)
    b_out = np.asarray(inputs["b_out"], np.float32)
    outs = np.stack([np.asarray(res.results[i]["out"]) for i in range(B)])
    out = outs.reshape(B, T, U, V) + b_out[None, None, None, :]
    return np.ascontiguousarray(out, dtype=np.float32), res


def kernel(**inputs):
    out, _ = _run(inputs)
    return out
